# revision 16
# baseline (speedup 1.0000x reference)
"""Trainium2 Bass kernel for a decoder block (MHA + GELU MLP, pre-LN, causal).

Problem shapes (hardcoded): B=2, T=2048, C=512, H=8, HD=64, f32 in/out.

Sharding: 8 cores = 2 batches x 4 cores. Causal work is balanced at
128-token query-tile granularity: core j of a batch owns q-tiles
{15-j, 11-j, 7-j, 3-j} (slots 0..3) with fixed per-slot key windows
W = [16, 12, 8, 4] tiles, so every core executes the identical program.
The key layout is a per-core host-side permutation of x's 128-row tiles
chosen so that (a) each slot's window prefix contains exactly the keys it
needs and (b) the diagonal (triangular-masked) tile always sits at the
window's last position. Per-(slot,position) exp biases (0 / -1e30) kill
the few padding tiles; queries come from a host-gathered copy of the
owned q-tiles.

All matmuls run in bf16 (fp32 PSUM accumulation): on TRN2 hardware f32r
matmuls stream 2 cycles/row while bf16 streams 1. Scores for all 8 heads
of one (slot, position) are written into one 2-bank PSUM tile so softmax
exp runs as a single [128,1024] activation. W1/W2 are staged whole in
SBUF so their DMA overlaps earlier phases. LN normalize + PSUM
evacuation run on the scalar engine to unload the vector engine.
"""

import os
import sys

for _p in ("/opt/trn_rl_repo",):
    if _p not in sys.path and os.path.isdir(_p):
        sys.path.insert(0, _p)

import ml_dtypes
import numpy as np

import concourse.bacc as bacc
import concourse.bass as bass
import concourse.tile as tile
from concourse import mybir
from concourse.bass_utils import run_bass_kernel_spmd

F32 = mybir.dt.float32
BF16 = mybir.dt.bfloat16
AF = mybir.ActivationFunctionType
NPBF = ml_dtypes.bfloat16

B, T, C, H, HD = 2, 2048, 512, 8, 64
NCORES = 8
QB = 512          # query tokens per core (4 tiles of 128)
NT = T // 128     # 16 key tiles
NQ = 4            # query tiles (slots) per core
NEG = -1.0e30

WIN = [16, 12, 8, 4]           # key-window tiles per slot
OFF = [0, 16, 28, 36]          # flat kbias column offset per slot
NW = sum(WIN)                  # 40

last_run = None       # test harness reads exec_time_ns from here
_prog_cache = {}


def _build_perm(j):
    """Key-tile permutation + owned q-tiles for core j (of 4)."""
    qs = [15 - j, 11 - j, 7 - j, 3 - j]
    pi = [None] * 16
    for s in range(4):
        pi[WIN[s] - 1] = qs[s]
    rest = iter([o for o in range(16) if o not in set(qs)])
    for p in range(16):
        if pi[p] is None:
            pi[p] = next(rest)
    return pi, qs


def _build_program(with_qkv_bias):
    nc = bacc.Bacc("TRN2", target_bir_lowering=False, debug=False,
                   num_devices=NCORES)

    xb_d = nc.dram_tensor("xb", [T, C], BF16, kind="ExternalInput")
    xq_d = nc.dram_tensor("xqg", [128, 4, 512], BF16, kind="ExternalInput")
    wq_d = nc.dram_tensor("wq", [128, 4, 512], BF16, kind="ExternalInput")
    wk_d = nc.dram_tensor("wk", [128, 4, 512], BF16, kind="ExternalInput")
    wv_d = nc.dram_tensor("wv", [128, 4, 512], BF16, kind="ExternalInput")
    wo_d = nc.dram_tensor("wo", [128, 4, 512], BF16, kind="ExternalInput")
    w1_d = nc.dram_tensor("w1", [128, 16, 512], BF16, kind="ExternalInput")
    w2_d = nc.dram_tensor("w2", [128, 16, 512], BF16, kind="ExternalInput")
    kb_d = nc.dram_tensor("kbias", [128, NW], F32, kind="ExternalInput")
    bo_d = nc.dram_tensor("bo", [1, 512], BF16, kind="ExternalInput")
    b1_d = nc.dram_tensor("b1c", [128, 16], F32, kind="ExternalInput")
    b2_d = nc.dram_tensor("b2r", [1, 512], BF16, kind="ExternalInput")
    id_d = nc.dram_tensor("identc", [128, 128], BF16, kind="ExternalInput")
    tr_d = nc.dram_tensor("tri8", [128, 2, 512], BF16, kind="ExternalInput")
    on_d = nc.dram_tensor("onesc", [128, 512], BF16, kind="ExternalInput")
    bq_d = (nc.dram_tensor("bqkv", [3, 1, 512], BF16, kind="ExternalInput")
            if with_qkv_bias else None)
    out_d = nc.dram_tensor("out", [QB, C], F32, kind="ExternalOutput")

    with tile.TileContext(nc) as tc:
        with (
            tc.tile_pool(name="const", bufs=1) as const,
            tc.tile_pool(name="mid", bufs=1) as mid,
            tc.tile_pool(name="tp", bufs=3) as tp,
            tc.tile_pool(name="sp", bufs=4) as sp,
        ):
            # ---------------- constants ----------------
            wo_sb = const.tile([128, 4, 512], BF16)
            nc.sync.dma_start(wo_sb[:], wo_d[:])
            kb_sb = const.tile([128, NW], F32)
            nc.sync.dma_start(kb_sb[:], kb_d[:])
            bo_sb = const.tile([1, 512], BF16)
            nc.sync.dma_start(bo_sb[:], bo_d[:])
            b1_sb = const.tile([128, 16], F32)
            nc.sync.dma_start(b1_sb[:], b1_d[:])
            b2_sb = const.tile([1, 512], BF16)
            nc.sync.dma_start(b2_sb[:], b2_d[:])
            if with_qkv_bias:
                bq_sb = const.tile([3, 1, 512], BF16)
                nc.sync.dma_start(bq_sb[:], bq_d[:])

            eps_sb = const.tile([128, 1], F32)
            nc.vector.memset(eps_sb[:], 1e-5)
            ones512 = const.tile([128, 512], BF16)
            nc.sync.dma_start(ones512[:], on_d[:])
            ones_sb = ones512  # [1, 128] slices come from row 0
            ident = const.tile([128, 128], BF16)
            nc.sync.dma_start(ident[:], id_d[:])
            tri_sb = const.tile([128, 2, 512], BF16)
            nc.sync.dma_start(tri_sb[:], tr_d[:])

            # ---------------- persistent mid tensors ----------------
            kt_sb = mid.tile([128, 4, 2048], BF16)   # K^T  (head pair, 64h+d)
            v_sb = mid.tile([128, 16, 520], BF16)    # V + ones column per head
            qt_sb = mid.tile([128, 4, 512], BF16)    # Q^T (slot-major columns)
            xq_sb = mid.tile([128, 4, 512], BF16)    # raw x of owned q-tiles
            nc.sync.dma_start(xq_sb[:], xq_d[:])
            # W1 / W2 staged whole: no deps, so DMA overlaps earlier phases
            w1_sb = mid.tile([128, 16, 512], BF16)
            nc.sync.dma_start(w1_sb[:], w1_d[:])
            w2_sb = mid.tile([128, 16, 512], BF16)
            nc.sync.dma_start(w2_sb[:], w2_d[:])
            # pre-set the ones columns (col 64 of each 65-wide head group)
            vones = (v_sb[:, :, :]
                     .rearrange("p a (h e) -> p a h e", e=65)[:, :, :, 64:65])
            nc.vector.tensor_copy(
                vones, ones512[:, 0:128]
                .rearrange("p (a h) -> p a h", h=8).unsqueeze(3))

            def layernorm_to(src_ap, dst_ap):
                st = sp.tile([128, 6], F32, tag="st")
                nc.vector.bn_stats(out=st[:], in_=src_ap)
                mv = sp.tile([128, 2], F32, tag="mv")
                nc.vector.bn_aggr(out=mv[:], in_=st[:])
                lg = sp.tile([128, 1], F32, tag="lg")
                nc.scalar.activation(out=lg[:], in_=mv[:, 1:2], func=AF.Sqrt,
                                     bias=eps_sb[:])
                rs = sp.tile([128, 1], F32, tag="rs")
                nc.vector.reciprocal(out=rs[:], in_=lg[:])
                nc.vector.tensor_scalar(
                    out=dst_ap, in0=src_ap, scalar1=mv[:, 0:1], scalar2=rs[:],
                    op0=mybir.AluOpType.subtract, op1=mybir.AluOpType.mult)

            # ======== phase 1+2 scope: LN1, transpose, Q/K/V ========
            with tc.tile_pool(name="p1", bufs=1) as p1:
                h1t_sb = p1.tile([128, 4, 2048], BF16)
                h1q_sb = p1.tile([128, 4, 512], BF16)
                wq_sb = p1.tile([128, 4, 512], BF16)
                nc.sync.dma_start(wq_sb[:], wq_d[:])
                wk_sb = p1.tile([128, 4, 512], BF16)
                nc.sync.dma_start(wk_sb[:], wk_d[:])
                wv_sb = p1.tile([128, 4, 512], BF16)
                nc.sync.dma_start(wv_sb[:], wv_d[:])

                with tc.tile_pool(name="ptr1", bufs=2, space="PSUM") as ptr1:
                    for t in range(NT):
                        xt = tp.tile([128, 512], BF16, tag="xt")
                        nc.sync.dma_start(xt[:], xb_d[bass.ts(t, 128), :])
                        ht = tp.tile([128, 512], BF16, tag="ht")
                        layernorm_to(xt[:], ht[:])
                        pst = ptr1.tile([128, 4, 128], BF16, tag="tr")
                        for cc in range(4):
                            nc.tensor.transpose(
                                pst[:, cc, :], ht[:, bass.ts(cc, 128)],
                                ident[:])
                        ev = h1t_sb[:, :, bass.ts(t, 128)]
                        nc.vector.tensor_copy(ev, pst[:])
                    # owned q-tiles: LN + transpose from the gathered copy
                    for qt in range(NQ):
                        ht = tp.tile([128, 512], BF16, tag="ht")
                        layernorm_to(xq_sb[:, qt, :], ht[:])
                        pst = ptr1.tile([128, 4, 128], BF16, tag="tr")
                        for cc in range(4):
                            nc.tensor.transpose(
                                pst[:, cc, :], ht[:, bass.ts(cc, 128)],
                                ident[:])
                        ev = h1q_sb[:, :, bass.ts(qt, 128)]
                        nc.vector.tensor_copy(ev, pst[:])

                with tc.tile_pool(name="pq", bufs=2, space="PSUM") as pq_ps:
                    # Q^T: head pairs; rhs = h1T of the owned q-tiles
                    for pr in range(4):
                        ps = pq_ps.tile([128, 512], F32, tag="ps")
                        for cc in range(4):
                            nc.tensor.matmul(
                                ps[:], wq_sb[:, cc, bass.ts(pr, 128)],
                                h1q_sb[:, cc, :],
                                start=(cc == 0),
                                stop=(cc == 3 and not with_qkv_bias))
                        if with_qkv_bias:
                            nc.tensor.matmul(
                                ps[:], bq_sb[0, :, bass.ts(pr, 128)],
                                ones512[:], start=False, stop=True)
                        nc.vector.tensor_copy(qt_sb[:, pr, :], ps[:])

                    # K^T: head pairs x 4 key chunks of 512
                    for pr in range(4):
                        for nk in range(4):
                            ps = pq_ps.tile([128, 512], F32, tag="ps")
                            for cc in range(4):
                                nc.tensor.matmul(
                                    ps[:], wk_sb[:, cc, bass.ts(pr, 128)],
                                    h1t_sb[:, cc, bass.ts(nk, 512)],
                                    start=(cc == 0),
                                    stop=(cc == 3 and not with_qkv_bias))
                            if with_qkv_bias:
                                nc.tensor.matmul(
                                    ps[:], bq_sb[1, :, bass.ts(pr, 128)],
                                    ones512[:], start=False, stop=True)
                            ev = kt_sb[:, pr, bass.ts(nk, 512)]
                            nc.vector.tensor_copy(ev, ps[:])

                    # V: 16 token tiles; rhs = all heads of Wv at once
                    for t in range(NT):
                        ps = pq_ps.tile([128, 512], F32, tag="ps")
                        for cc in range(4):
                            nc.tensor.matmul(
                                ps[:], h1t_sb[:, cc, bass.ts(t, 128)],
                                wv_sb[:, cc, :],
                                start=(cc == 0),
                                stop=(cc == 3 and not with_qkv_bias))
                        if with_qkv_bias:
                            nc.tensor.matmul(
                                ps[:], ones_sb[0:1, 0:128], bq_sb[2],
                                start=False, stop=True)
                        ev = (v_sb[:, t, :]
                              .rearrange("p (h e) -> p h e", e=65)[:, :, 0:64])
                        sv = ps[:].rearrange("p (h e) -> p h e", e=64)
                        nc.vector.tensor_copy(ev, sv)

            # ======== phases 3..7 scope ========
            with tc.tile_pool(name="mid2", bufs=1) as mid2:
                at_sb = mid2.tile([128, 4, 512], BF16)   # attnT (scaled)
                x2_sb = mid2.tile([128, 4, 512], F32)    # post-attn residual
                h2t_sb = mid2.tile([128, 4, 512], BF16)  # ln2(x2)^T
                g_sb = mid2.tile([128, 16, 512], BF16)   # gelu(ffn1)^T

                # -------- phase 3: attention (slot-window scheme) --------
                # pss column sub-tile index hpos = half*4 + pr; actual head
                # for v_sb / weight layouts is h = 2*pr + half.
                with (
                    tc.tile_pool(name="psS", bufs=2, space="PSUM") as ps_ps,
                    tc.tile_pool(name="psO", bufs=2, space="PSUM") as po_ps,
                    tc.tile_pool(name="ap", bufs=4) as ap_pool,
                    tc.tile_pool(name="drp", bufs=2, space="DRAM") as drp,
                ):
                    for s in range(NQ):
                        po = po_ps.tile([65, 8, 128], F32, tag="po")
                        for w in range(WIN[s]):
                            last = (w == WIN[s] - 1)
                            pss = ps_ps.tile([128, 2, 512], F32, tag="ps")
                            psf = pss[:].rearrange("p a n -> p (a n)")
                            for hpos in range(8):
                                half, pr = hpos // 4, hpos % 4
                                base = 64 * half
                                nc.tensor.matmul(
                                    psf[:, bass.ts(hpos, 128)],
                                    kt_sb[base:base + 64, pr, bass.ts(w, 128)],
                                    qt_sb[base:base + 64, pr, bass.ts(s, 128)],
                                    start=True, stop=True,
                                    skip_group_check=True)
                            ptile = ap_pool.tile([128, 2, 512], BF16,
                                                 tag="pt")
                            col = OFF[s] + w
                            nc.scalar.activation(
                                out=ptile[:], in_=pss[:], func=AF.Exp,
                                bias=kb_sb[:, col:col + 1])
                            if last:
                                # diagonal tile: zero the strictly-upper
                                # triangle after exp (0/1 mask, all heads)
                                ptm = ap_pool.tile([128, 2, 512], BF16,
                                                   tag="ptm")
                                nc.vector.tensor_mul(
                                    out=ptm[:], in0=ptile[:], in1=tri_sb[:])
                                ptile = ptm
                            ptf = ptile[:].rearrange("p a n -> p (a n)")
                            for hpos in range(8):
                                half, pr = hpos // 4, hpos % 4
                                h = 2 * pr + half
                                nc.tensor.matmul(
                                    po[:, hpos, :],
                                    v_sb[:, w, h * 65:(h + 1) * 65],
                                    ptf[:, bass.ts(hpos, 128)],
                                    start=(w == 0), stop=last,
                                    skip_group_check=True)
                        # normalize: denom row 64 -> broadcast -> multiply
                        rec = ap_pool.tile([1, 8, 128], F32, tag="rec")
                        nc.vector.reciprocal(out=rec[:], in_=po[64:65, :, :])
                        rd = drp.tile([1, 1024], F32, tag="rd")
                        nc.sync.dma_start(
                            rd[:], rec[:].rearrange("p a n -> p (a n)"))
                        rb = ap_pool.tile([64, 8, 128], F32, tag="rb")
                        nc.sync.dma_start(
                            rb[:].rearrange("p a n -> p (a n)"),
                            rd[:].to_broadcast([64, 1024]))
                        for half in range(2):
                            base = 64 * half
                            nc.vector.tensor_mul(
                                out=at_sb[base:base + 64, :,
                                          bass.ts(s, 128)],
                                in0=po[0:64, 4 * half:4 * half + 4, :],
                                in1=rb[:, 4 * half:4 * half + 4, :])

                with tc.tile_pool(name="pf", bufs=3, space="PSUM") as pf_ps:
                    # -------- phase 4: output projection + residual --------
                    for qt in range(NQ):
                        ps = pf_ps.tile([128, 512], F32, tag="pf")
                        for cc in range(4):
                            nc.tensor.matmul(
                                ps[:], at_sb[:, cc, bass.ts(qt, 128)],
                                wo_sb[:, cc, :], start=(cc == 0), stop=False)
                        nc.tensor.matmul(ps[:], ones_sb[0:1, 0:128],
                                         bo_sb[:], start=False, stop=True)
                        nc.vector.tensor_add(out=x2_sb[:, qt, :], in0=ps[:],
                                             in1=xq_sb[:, qt, :])

                    # -------- phase 5: LN2 + transpose --------
                    with tc.tile_pool(name="ptr2", bufs=2,
                                      space="PSUM") as ptr2:
                        for qt in range(NQ):
                            ht = tp.tile([128, 512], BF16, tag="ht")
                            layernorm_to(x2_sb[:, qt, :], ht[:])
                            pst = ptr2.tile([128, 4, 128], BF16, tag="tr")
                            for cc in range(4):
                                nc.tensor.transpose(
                                    pst[:, cc, :],
                                    ht[:, bass.ts(cc, 128)], ident[:])
                            ev = h2t_sb[:, :, bass.ts(qt, 128)]
                            nc.vector.tensor_copy(ev, pst[:])

                    # -------- phase 6: FFN1 + gelu --------
                    for f in range(16):
                        ps = pf_ps.tile([128, 512], F32, tag="pf")
                        for cc in range(4):
                            nc.tensor.matmul(
                                ps[:], w1_sb[:, f, bass.ts(cc, 128)],
                                h2t_sb[:, cc, :],
                                start=(cc == 0), stop=(cc == 3))
                        nc.scalar.activation(
                            out=g_sb[:, f, :], in_=ps[:], func=AF.Gelu,
                            bias=b1_sb[:, f:f + 1])

                    # -------- phase 7: FFN2 + residual + store --------
                    with (
                        tc.tile_pool(name="pf2", bufs=1,
                                     space="PSUM") as pf2_ps,
                        tc.tile_pool(name="op", bufs=2) as op,
                    ):
                        pso = [pf2_ps.tile([128, 512], F32, tag=f"o{qt}",
                                           name=f"o{qt}") for qt in range(NQ)]
                        for ff in range(16):
                            for qt in range(NQ):
                                nc.tensor.matmul(
                                    pso[qt][:],
                                    g_sb[:, ff, bass.ts(qt, 128)],
                                    w2_sb[:, ff, :], start=(ff == 0),
                                    stop=False, skip_group_check=True)
                        for qt in range(NQ):
                            nc.tensor.matmul(
                                pso[qt][:], ones_sb[0:1, 0:128], b2_sb[:],
                                start=False, stop=True, skip_group_check=True)
                            ot = op.tile([128, 512], F32, tag="ot")
                            nc.vector.tensor_add(out=ot[:], in0=pso[qt][:],
                                                 in1=x2_sb[:, qt, :])
                            nc.sync.dma_start(out_d[bass.ts(qt, 128), :], ot[:])

    nc.compile()
    return nc


def _bf(a):
    return np.ascontiguousarray(np.asarray(a, np.float32).astype(NPBF))


def _host_prep(x, Wq, Wk, Wv, Wo, bo, W1, b1, W2, b2, g1, be1, g2, be2):
    """Fold LN gains into weights; build per-core permuted inputs/biases."""
    x = np.asarray(x, np.float32)
    g1 = np.asarray(g1, np.float32)
    be1 = np.asarray(be1, np.float32)
    g2 = np.asarray(g2, np.float32)
    be2 = np.asarray(be2, np.float32)

    wq_cat = np.transpose(np.asarray(Wq, np.float32), (1, 0, 2)).reshape(C, H * HD)
    wk_cat = np.transpose(np.asarray(Wk, np.float32), (1, 0, 2)).reshape(C, H * HD)
    wv_cat = np.transpose(np.asarray(Wv, np.float32), (1, 0, 2)).reshape(C, H * HD)
    scl = float(HD) ** -0.5
    wq_f = (g1[:, None] * wq_cat) * scl
    wk_f = g1[:, None] * wk_cat
    wv_f = g1[:, None] * wv_cat
    bq = (be1 @ wq_cat) * scl
    bk = be1 @ wk_cat
    bv = be1 @ wv_cat
    with_qkv_bias = bool(np.any(bq) or np.any(bk) or np.any(bv))

    W1 = np.asarray(W1, np.float32)
    w1_f = g2[:, None] * W1
    b1_f = np.asarray(b1, np.float32) + be2 @ W1

    # 0/1 keep-mask for the diagonal tile, S^T layout, tiled for 8 heads
    tri = (np.arange(128)[:, None] <= np.arange(128)[None, :]) \
        .astype(np.float32)
    tri8 = np.tile(tri, (1, 8)).reshape(128, 2, 512)

    common = {
        "identc": _bf(np.eye(128, dtype=np.float32)),
        "tri8": _bf(tri8),
        "onesc": _bf(np.ones((128, 512), np.float32)),
        "wq": _bf(wq_f.reshape(4, 128, 512).transpose(1, 0, 2)),
        "wk": _bf(wk_f.reshape(4, 128, 512).transpose(1, 0, 2)),
        "wv": _bf(wv_f.reshape(4, 128, 512).transpose(1, 0, 2)),
        "wo": _bf(
            np.asarray(Wo, np.float32).reshape(4, 128, 512).transpose(1, 0, 2)),
        "w1": _bf(
            w1_f.reshape(4, 128, 16, 128).transpose(1, 2, 0, 3)
            .reshape(128, 16, 512)),
        "w2": _bf(np.asarray(W2, np.float32).reshape(16, 128, 512)
                  .transpose(1, 0, 2)),
        "bo": _bf(np.asarray(bo, np.float32).reshape(1, 512)),
        "b1c": np.ascontiguousarray(b1_f.reshape(16, 128).T),
        "b2r": _bf(np.asarray(b2, np.float32).reshape(1, 512)),
    }
    if with_qkv_bias:
        common["bqkv"] = _bf(np.stack([bq, bk, bv]).reshape(3, 1, 512))

    in_maps = []
    perms = []
    for c in range(NCORES):
        bb, j = c // 4, c % 4
        pi, qs = _build_perm(j)
        perms.append(qs)
        xt = x[bb].reshape(16, 128, C)
        xb_perm = xt[pi].reshape(T, C)
        xqg = np.ascontiguousarray(
            xt[qs].transpose(1, 0, 2))            # [128, 4, 512]
        kbias = np.zeros(NW, np.float32)
        for s in range(4):
            for w in range(WIN[s] - 1):
                if pi[w] > qs[s]:
                    kbias[OFF[s] + w] = NEG
        im = dict(common)
        im["xb"] = _bf(xb_perm)
        im["xqg"] = _bf(xqg)
        im["kbias"] = np.ascontiguousarray(
            np.broadcast_to(kbias.reshape(1, NW), (128, NW)))
        in_maps.append(im)
    return in_maps, with_qkv_bias, perms


def kernel(**inputs):
    global last_run
    in_maps, with_qkv_bias, perms = _host_prep(**inputs)
    if with_qkv_bias not in _prog_cache:
        _prog_cache[with_qkv_bias] = _build_program(with_qkv_bias)
    nc = _prog_cache[with_qkv_bias]
    res = run_bass_kernel_spmd(nc, in_maps, list(range(NCORES)))
    last_run = res
    out = np.empty((B, T, C), np.float32)
    for c in range(NCORES):
        bb = c // 4
        r = np.asarray(res.results[c]["out"])
        for s in range(4):
            qt = perms[c][s]
            out[bb, 128 * qt:128 * (qt + 1), :] = r[128 * s:128 * (s + 1), :]
    return out


# revision 17
# speedup vs baseline: 1.0144x; 1.0144x over previous
"""Trainium2 Bass kernel for a decoder block (MHA + GELU MLP, pre-LN, causal).

Problem shapes (hardcoded): B=2, T=2048, C=512, H=8, HD=64, f32 in/out.

Sharding: 8 cores = 2 batches x 4 cores. Causal work is balanced at
128-token query-tile granularity: core j of a batch owns q-tiles
{15-j, 11-j, 7-j, 3-j} (slots 0..3) with fixed per-slot key windows
W = [16, 12, 8, 4] tiles, so every core executes the identical program.
The key layout is a per-core host-side permutation of x's 128-row tiles
chosen so that (a) each slot's window prefix contains exactly the keys it
needs and (b) the diagonal (triangular-masked) tile always sits at the
window's last position. Per-(slot,position) exp biases (0 / -1e30) kill
the few padding tiles; queries come from a host-gathered copy of the
owned q-tiles.

All matmuls run in bf16 (fp32 PSUM accumulation): on TRN2 hardware f32r
matmuls stream 2 cycles/row while bf16 streams 1. Scores for all 8 heads
of one (slot, position) are written into one 2-bank PSUM tile so softmax
exp runs as a single [128,1024] activation. W1/W2 are staged whole in
SBUF so their DMA overlaps earlier phases. LN normalize + PSUM
evacuation run on the scalar engine to unload the vector engine.
"""

import os
import sys

for _p in ("/opt/trn_rl_repo",):
    if _p not in sys.path and os.path.isdir(_p):
        sys.path.insert(0, _p)

import ml_dtypes
import numpy as np

import concourse.bacc as bacc
import concourse.bass as bass
import concourse.tile as tile
from concourse import mybir
from concourse.bass_utils import run_bass_kernel_spmd

F32 = mybir.dt.float32
BF16 = mybir.dt.bfloat16
AF = mybir.ActivationFunctionType
NPBF = ml_dtypes.bfloat16

B, T, C, H, HD = 2, 2048, 512, 8, 64
NCORES = 8
QB = 512          # query tokens per core (4 tiles of 128)
NT = T // 128     # 16 key tiles
NQ = 4            # query tiles (slots) per core
NEG = -1.0e30

WIN = [16, 12, 8, 4]           # key-window tiles per slot
OFF = [0, 16, 28, 36]          # flat kbias column offset per slot
NW = sum(WIN)                  # 40

last_run = None       # test harness reads exec_time_ns from here
_prog_cache = {}


def _build_perm(j):
    """Key-tile permutation + owned q-tiles for core j (of 4)."""
    qs = [15 - j, 11 - j, 7 - j, 3 - j]
    pi = [None] * 16
    for s in range(4):
        pi[WIN[s] - 1] = qs[s]
    rest = iter([o for o in range(16) if o not in set(qs)])
    for p in range(16):
        if pi[p] is None:
            pi[p] = next(rest)
    return pi, qs


def _build_program(with_qkv_bias):
    nc = bacc.Bacc("TRN2", target_bir_lowering=False, debug=False,
                   num_devices=NCORES)

    xb_d = nc.dram_tensor("xb", [T, C], BF16, kind="ExternalInput")
    xq_d = nc.dram_tensor("xqg", [128, 4, 512], BF16, kind="ExternalInput")
    wq_d = nc.dram_tensor("wq", [128, 4, 512], BF16, kind="ExternalInput")
    wk_d = nc.dram_tensor("wk", [128, 4, 512], BF16, kind="ExternalInput")
    wv_d = nc.dram_tensor("wv", [128, 4, 512], BF16, kind="ExternalInput")
    wo_d = nc.dram_tensor("wo", [128, 4, 512], BF16, kind="ExternalInput")
    w1_d = nc.dram_tensor("w1", [128, 16, 512], BF16, kind="ExternalInput")
    w2_d = nc.dram_tensor("w2", [128, 16, 512], BF16, kind="ExternalInput")
    kb_d = nc.dram_tensor("kbias", [128, NW], F32, kind="ExternalInput")
    bo_d = nc.dram_tensor("bo", [1, 512], BF16, kind="ExternalInput")
    b1_d = nc.dram_tensor("b1c", [128, 16], F32, kind="ExternalInput")
    b2_d = nc.dram_tensor("b2r", [1, 512], BF16, kind="ExternalInput")
    id_d = nc.dram_tensor("identc", [128, 128], BF16, kind="ExternalInput")
    tr_d = nc.dram_tensor("tri8", [128, 2, 512], BF16, kind="ExternalInput")
    on_d = nc.dram_tensor("onesc", [128, 512], BF16, kind="ExternalInput")
    bq_d = (nc.dram_tensor("bqkv", [3, 1, 512], BF16, kind="ExternalInput")
            if with_qkv_bias else None)
    out_d = nc.dram_tensor("out", [QB, C], F32, kind="ExternalOutput")

    with tile.TileContext(nc) as tc:
        with (
            tc.tile_pool(name="const", bufs=1) as const,
            tc.tile_pool(name="mid", bufs=1) as mid,
            tc.tile_pool(name="tp", bufs=3) as tp,
            tc.tile_pool(name="sp", bufs=4) as sp,
        ):
            # ---------------- constants ----------------
            wo_sb = const.tile([128, 4, 512], BF16)
            nc.sync.dma_start(wo_sb[:], wo_d[:])
            kb_sb = const.tile([128, NW], F32)
            nc.sync.dma_start(kb_sb[:], kb_d[:])
            bo_sb = const.tile([1, 512], BF16)
            nc.sync.dma_start(bo_sb[:], bo_d[:])
            b1_sb = const.tile([128, 16], F32)
            nc.sync.dma_start(b1_sb[:], b1_d[:])
            b2_sb = const.tile([1, 512], BF16)
            nc.sync.dma_start(b2_sb[:], b2_d[:])
            if with_qkv_bias:
                bq_sb = const.tile([3, 1, 512], BF16)
                nc.sync.dma_start(bq_sb[:], bq_d[:])

            eps_sb = const.tile([128, 1], F32)
            nc.vector.memset(eps_sb[:], 1e-5)
            ones512 = const.tile([128, 512], BF16)
            nc.sync.dma_start(ones512[:], on_d[:])
            ones_sb = ones512  # [1, 128] slices come from row 0
            ident = const.tile([128, 128], BF16)
            nc.sync.dma_start(ident[:], id_d[:])
            tri_sb = const.tile([128, 2, 512], BF16)
            nc.sync.dma_start(tri_sb[:], tr_d[:])

            # ---------------- persistent mid tensors ----------------
            kt_sb = mid.tile([128, 4, 2048], BF16)   # K^T  (head pair, 64h+d)
            v_sb = mid.tile([128, 16, 520], BF16)    # V + ones column per head
            qt_sb = mid.tile([128, 4, 512], BF16)    # Q^T (slot-major columns)
            xq_sb = mid.tile([128, 4, 512], BF16)    # raw x of owned q-tiles
            nc.sync.dma_start(xq_sb[:], xq_d[:])
            # W1 / W2 staged whole: no deps, so DMA overlaps earlier phases
            w1_sb = mid.tile([128, 16, 512], BF16)
            nc.sync.dma_start(w1_sb[:], w1_d[:])
            w2_sb = mid.tile([128, 16, 512], BF16)
            nc.sync.dma_start(w2_sb[:], w2_d[:])
            # pre-set the ones columns (col 64 of each 65-wide head group)
            vones = (v_sb[:, :, :]
                     .rearrange("p a (h e) -> p a h e", e=65)[:, :, :, 64:65])
            nc.vector.tensor_copy(
                vones, ones512[:, 0:128]
                .rearrange("p (a h) -> p a h", h=8).unsqueeze(3))

            def layernorm_to(src_ap, dst_ap):
                st = sp.tile([128, 6], F32, tag="st")
                nc.vector.bn_stats(out=st[:], in_=src_ap)
                mv = sp.tile([128, 2], F32, tag="mv")
                nc.vector.bn_aggr(out=mv[:], in_=st[:])
                lg = sp.tile([128, 1], F32, tag="lg")
                nc.scalar.activation(out=lg[:], in_=mv[:, 1:2], func=AF.Sqrt,
                                     bias=eps_sb[:])
                rs = sp.tile([128, 1], F32, tag="rs")
                nc.vector.reciprocal(out=rs[:], in_=lg[:])
                nc.vector.tensor_scalar(
                    out=dst_ap, in0=src_ap, scalar1=mv[:, 0:1], scalar2=rs[:],
                    op0=mybir.AluOpType.subtract, op1=mybir.AluOpType.mult)

            # ======== phase 1+2 scope: LN1, transpose, Q/K/V ========
            with tc.tile_pool(name="p1", bufs=1) as p1:
                h1t_sb = p1.tile([128, 4, 2048], BF16)
                h1q_sb = p1.tile([128, 4, 512], BF16)
                wq_sb = p1.tile([128, 4, 512], BF16)
                nc.sync.dma_start(wq_sb[:], wq_d[:])
                wk_sb = p1.tile([128, 4, 512], BF16)
                nc.sync.dma_start(wk_sb[:], wk_d[:])
                wv_sb = p1.tile([128, 4, 512], BF16)
                nc.sync.dma_start(wv_sb[:], wv_d[:])

                with tc.tile_pool(name="ptr1", bufs=2, space="PSUM") as ptr1:
                    for t in range(NT):
                        xt = tp.tile([128, 512], BF16, tag="xt")
                        nc.sync.dma_start(xt[:], xb_d[bass.ts(t, 128), :])
                        ht = tp.tile([128, 512], BF16, tag="ht")
                        layernorm_to(xt[:], ht[:])
                        pst = ptr1.tile([128, 4, 128], BF16, tag="tr")
                        for cc in range(4):
                            nc.tensor.transpose(
                                pst[:, cc, :], ht[:, bass.ts(cc, 128)],
                                ident[:])
                        ev = h1t_sb[:, :, bass.ts(t, 128)]
                        nc.vector.tensor_copy(ev, pst[:])
                    # owned q-tiles: LN + transpose from the gathered copy
                    for qt in range(NQ):
                        ht = tp.tile([128, 512], BF16, tag="ht")
                        layernorm_to(xq_sb[:, qt, :], ht[:])
                        pst = ptr1.tile([128, 4, 128], BF16, tag="tr")
                        for cc in range(4):
                            nc.tensor.transpose(
                                pst[:, cc, :], ht[:, bass.ts(cc, 128)],
                                ident[:])
                        ev = h1q_sb[:, :, bass.ts(qt, 128)]
                        nc.vector.tensor_copy(ev, pst[:])

                with tc.tile_pool(name="pq", bufs=2, space="PSUM") as pq_ps:
                    # Q^T: head pairs; rhs = h1T of the owned q-tiles
                    for pr in range(4):
                        ps = pq_ps.tile([128, 512], F32, tag="ps")
                        for cc in range(4):
                            nc.tensor.matmul(
                                ps[:], wq_sb[:, cc, bass.ts(pr, 128)],
                                h1q_sb[:, cc, :],
                                start=(cc == 0),
                                stop=(cc == 3 and not with_qkv_bias))
                        if with_qkv_bias:
                            nc.tensor.matmul(
                                ps[:], bq_sb[0, :, bass.ts(pr, 128)],
                                ones512[:], start=False, stop=True)
                        nc.vector.tensor_copy(qt_sb[:, pr, :], ps[:])

                    # K^T: head pairs x 4 key chunks of 512
                    for pr in range(4):
                        for nk in range(4):
                            ps = pq_ps.tile([128, 512], F32, tag="ps")
                            for cc in range(4):
                                nc.tensor.matmul(
                                    ps[:], wk_sb[:, cc, bass.ts(pr, 128)],
                                    h1t_sb[:, cc, bass.ts(nk, 512)],
                                    start=(cc == 0),
                                    stop=(cc == 3 and not with_qkv_bias))
                            if with_qkv_bias:
                                nc.tensor.matmul(
                                    ps[:], bq_sb[1, :, bass.ts(pr, 128)],
                                    ones512[:], start=False, stop=True)
                            ev = kt_sb[:, pr, bass.ts(nk, 512)]
                            nc.vector.tensor_copy(ev, ps[:])

                    # V: 16 token tiles; rhs = all heads of Wv at once
                    for t in range(NT):
                        ps = pq_ps.tile([128, 512], F32, tag="ps")
                        for cc in range(4):
                            nc.tensor.matmul(
                                ps[:], h1t_sb[:, cc, bass.ts(t, 128)],
                                wv_sb[:, cc, :],
                                start=(cc == 0),
                                stop=(cc == 3 and not with_qkv_bias))
                        if with_qkv_bias:
                            nc.tensor.matmul(
                                ps[:], ones_sb[0:1, 0:128], bq_sb[2],
                                start=False, stop=True)
                        ev = (v_sb[:, t, :]
                              .rearrange("p (h e) -> p h e", e=65)[:, :, 0:64])
                        sv = ps[:].rearrange("p (h e) -> p h e", e=64)
                        nc.vector.tensor_copy(ev, sv)

            # ======== phases 3..7 scope ========
            with tc.tile_pool(name="mid2", bufs=1) as mid2:
                at_sb = mid2.tile([128, 4, 512], BF16)   # attnT (scaled)
                x2_sb = mid2.tile([128, 4, 512], F32)    # post-attn residual
                h2t_sb = mid2.tile([128, 4, 512], BF16)  # ln2(x2)^T
                g_sb = mid2.tile([128, 16, 512], BF16)   # gelu(ffn1)^T

                # -------- phase 3: attention (slot-window scheme) --------
                # pss column sub-tile index hpos = half*4 + pr; actual head
                # for v_sb / weight layouts is h = 2*pr + half.
                with (
                    tc.tile_pool(name="psS", bufs=2, space="PSUM") as ps_ps,
                    tc.tile_pool(name="psO", bufs=2, space="PSUM") as po_ps,
                    tc.tile_pool(name="ap", bufs=4) as ap_pool,
                    tc.tile_pool(name="drp", bufs=2, space="DRAM") as drp,
                ):
                    for s in range(NQ):
                        po = po_ps.tile([65, 8, 128], F32, tag="po")
                        for w in range(WIN[s]):
                            last = (w == WIN[s] - 1)
                            pss = ps_ps.tile([128, 2, 512], F32, tag="ps")
                            psf = pss[:].rearrange("p a n -> p (a n)")
                            for hpos in range(8):
                                half, pr = hpos // 4, hpos % 4
                                base = 64 * half
                                nc.tensor.matmul(
                                    psf[:, bass.ts(hpos, 128)],
                                    kt_sb[base:base + 64, pr, bass.ts(w, 128)],
                                    qt_sb[base:base + 64, pr, bass.ts(s, 128)],
                                    start=True, stop=True)
                            ptile = ap_pool.tile([128, 2, 512], BF16,
                                                 tag="pt")
                            col = OFF[s] + w
                            nc.scalar.activation(
                                out=ptile[:], in_=pss[:], func=AF.Exp,
                                bias=kb_sb[:, col:col + 1])
                            if last:
                                # diagonal tile: zero the strictly-upper
                                # triangle after exp (0/1 mask, all heads)
                                ptm = ap_pool.tile([128, 2, 512], BF16,
                                                   tag="ptm")
                                nc.vector.tensor_mul(
                                    out=ptm[:], in0=ptile[:], in1=tri_sb[:])
                                ptile = ptm
                            ptf = ptile[:].rearrange("p a n -> p (a n)")
                            for hpos in range(8):
                                half, pr = hpos // 4, hpos % 4
                                h = 2 * pr + half
                                nc.tensor.matmul(
                                    po[:, hpos, :],
                                    v_sb[:, w, h * 65:(h + 1) * 65],
                                    ptf[:, bass.ts(hpos, 128)],
                                    start=(w == 0), stop=last,
                                    skip_group_check=True)
                        # normalize: denom row 64 -> broadcast -> multiply
                        rec = ap_pool.tile([1, 8, 128], F32, tag="rec")
                        nc.vector.reciprocal(out=rec[:], in_=po[64:65, :, :])
                        rd = drp.tile([1, 1024], F32, tag="rd")
                        nc.sync.dma_start(
                            rd[:], rec[:].rearrange("p a n -> p (a n)"))
                        rb = ap_pool.tile([64, 8, 128], F32, tag="rb")
                        nc.sync.dma_start(
                            rb[:].rearrange("p a n -> p (a n)"),
                            rd[:].to_broadcast([64, 1024]))
                        for half in range(2):
                            base = 64 * half
                            nc.vector.tensor_mul(
                                out=at_sb[base:base + 64, :,
                                          bass.ts(s, 128)],
                                in0=po[0:64, 4 * half:4 * half + 4, :],
                                in1=rb[:, 4 * half:4 * half + 4, :])

                with tc.tile_pool(name="pf", bufs=3, space="PSUM") as pf_ps:
                    # -------- phase 4: output projection + residual --------
                    for qt in range(NQ):
                        ps = pf_ps.tile([128, 512], F32, tag="pf")
                        for cc in range(4):
                            nc.tensor.matmul(
                                ps[:], at_sb[:, cc, bass.ts(qt, 128)],
                                wo_sb[:, cc, :], start=(cc == 0), stop=False)
                        nc.tensor.matmul(ps[:], ones_sb[0:1, 0:128],
                                         bo_sb[:], start=False, stop=True)
                        nc.vector.tensor_add(out=x2_sb[:, qt, :], in0=ps[:],
                                             in1=xq_sb[:, qt, :])

                    # -------- phase 5: LN2 + transpose --------
                    with tc.tile_pool(name="ptr2", bufs=2,
                                      space="PSUM") as ptr2:
                        for qt in range(NQ):
                            ht = tp.tile([128, 512], BF16, tag="ht")
                            layernorm_to(x2_sb[:, qt, :], ht[:])
                            pst = ptr2.tile([128, 4, 128], BF16, tag="tr")
                            for cc in range(4):
                                nc.tensor.transpose(
                                    pst[:, cc, :],
                                    ht[:, bass.ts(cc, 128)], ident[:])
                            ev = h2t_sb[:, :, bass.ts(qt, 128)]
                            nc.vector.tensor_copy(ev, pst[:])

                    # -------- phase 6: FFN1 + gelu --------
                    for f in range(16):
                        ps = pf_ps.tile([128, 512], F32, tag="pf")
                        for cc in range(4):
                            nc.tensor.matmul(
                                ps[:], w1_sb[:, f, bass.ts(cc, 128)],
                                h2t_sb[:, cc, :],
                                start=(cc == 0), stop=(cc == 3))
                        nc.scalar.activation(
                            out=g_sb[:, f, :], in_=ps[:], func=AF.Gelu,
                            bias=b1_sb[:, f:f + 1])

                    # -------- phase 7: FFN2 + residual + store --------
                    with (
                        tc.tile_pool(name="pf2", bufs=1,
                                     space="PSUM") as pf2_ps,
                        tc.tile_pool(name="op", bufs=2) as op,
                    ):
                        pso = [pf2_ps.tile([128, 512], F32, tag=f"o{qt}",
                                           name=f"o{qt}") for qt in range(NQ)]
                        for ff in range(16):
                            for qt in range(NQ):
                                nc.tensor.matmul(
                                    pso[qt][:],
                                    g_sb[:, ff, bass.ts(qt, 128)],
                                    w2_sb[:, ff, :], start=(ff == 0),
                                    stop=False, skip_group_check=True)
                        for qt in range(NQ):
                            nc.tensor.matmul(
                                pso[qt][:], ones_sb[0:1, 0:128], b2_sb[:],
                                start=False, stop=True, skip_group_check=True)
                            ot = op.tile([128, 512], F32, tag="ot")
                            nc.vector.tensor_add(out=ot[:], in0=pso[qt][:],
                                                 in1=x2_sb[:, qt, :])
                            nc.sync.dma_start(out_d[bass.ts(qt, 128), :], ot[:])

    nc.compile()
    return nc


def _bf(a):
    return np.ascontiguousarray(np.asarray(a, np.float32).astype(NPBF))


def _host_prep(x, Wq, Wk, Wv, Wo, bo, W1, b1, W2, b2, g1, be1, g2, be2):
    """Fold LN gains into weights; build per-core permuted inputs/biases."""
    x = np.asarray(x, np.float32)
    g1 = np.asarray(g1, np.float32)
    be1 = np.asarray(be1, np.float32)
    g2 = np.asarray(g2, np.float32)
    be2 = np.asarray(be2, np.float32)

    wq_cat = np.transpose(np.asarray(Wq, np.float32), (1, 0, 2)).reshape(C, H * HD)
    wk_cat = np.transpose(np.asarray(Wk, np.float32), (1, 0, 2)).reshape(C, H * HD)
    wv_cat = np.transpose(np.asarray(Wv, np.float32), (1, 0, 2)).reshape(C, H * HD)
    scl = float(HD) ** -0.5
    wq_f = (g1[:, None] * wq_cat) * scl
    wk_f = g1[:, None] * wk_cat
    wv_f = g1[:, None] * wv_cat
    bq = (be1 @ wq_cat) * scl
    bk = be1 @ wk_cat
    bv = be1 @ wv_cat
    with_qkv_bias = bool(np.any(bq) or np.any(bk) or np.any(bv))

    W1 = np.asarray(W1, np.float32)
    w1_f = g2[:, None] * W1
    b1_f = np.asarray(b1, np.float32) + be2 @ W1

    # 0/1 keep-mask for the diagonal tile, S^T layout, tiled for 8 heads
    tri = (np.arange(128)[:, None] <= np.arange(128)[None, :]) \
        .astype(np.float32)
    tri8 = np.tile(tri, (1, 8)).reshape(128, 2, 512)

    common = {
        "identc": _bf(np.eye(128, dtype=np.float32)),
        "tri8": _bf(tri8),
        "onesc": _bf(np.ones((128, 512), np.float32)),
        "wq": _bf(wq_f.reshape(4, 128, 512).transpose(1, 0, 2)),
        "wk": _bf(wk_f.reshape(4, 128, 512).transpose(1, 0, 2)),
        "wv": _bf(wv_f.reshape(4, 128, 512).transpose(1, 0, 2)),
        "wo": _bf(
            np.asarray(Wo, np.float32).reshape(4, 128, 512).transpose(1, 0, 2)),
        "w1": _bf(
            w1_f.reshape(4, 128, 16, 128).transpose(1, 2, 0, 3)
            .reshape(128, 16, 512)),
        "w2": _bf(np.asarray(W2, np.float32).reshape(16, 128, 512)
                  .transpose(1, 0, 2)),
        "bo": _bf(np.asarray(bo, np.float32).reshape(1, 512)),
        "b1c": np.ascontiguousarray(b1_f.reshape(16, 128).T),
        "b2r": _bf(np.asarray(b2, np.float32).reshape(1, 512)),
    }
    if with_qkv_bias:
        common["bqkv"] = _bf(np.stack([bq, bk, bv]).reshape(3, 1, 512))

    in_maps = []
    perms = []
    for c in range(NCORES):
        bb, j = c // 4, c % 4
        pi, qs = _build_perm(j)
        perms.append(qs)
        xt = x[bb].reshape(16, 128, C)
        xb_perm = xt[pi].reshape(T, C)
        xqg = np.ascontiguousarray(
            xt[qs].transpose(1, 0, 2))            # [128, 4, 512]
        kbias = np.zeros(NW, np.float32)
        for s in range(4):
            for w in range(WIN[s] - 1):
                if pi[w] > qs[s]:
                    kbias[OFF[s] + w] = NEG
        im = dict(common)
        im["xb"] = _bf(xb_perm)
        im["xqg"] = _bf(xqg)
        im["kbias"] = np.ascontiguousarray(
            np.broadcast_to(kbias.reshape(1, NW), (128, NW)))
        in_maps.append(im)
    return in_maps, with_qkv_bias, perms


def kernel(**inputs):
    global last_run
    in_maps, with_qkv_bias, perms = _host_prep(**inputs)
    if with_qkv_bias not in _prog_cache:
        _prog_cache[with_qkv_bias] = _build_program(with_qkv_bias)
    nc = _prog_cache[with_qkv_bias]
    res = run_bass_kernel_spmd(nc, in_maps, list(range(NCORES)))
    last_run = res
    out = np.empty((B, T, C), np.float32)
    for c in range(NCORES):
        bb = c // 4
        r = np.asarray(res.results[c]["out"])
        for s in range(4):
            qt = perms[c][s]
            out[bb, 128 * qt:128 * (qt + 1), :] = r[128 * s:128 * (s + 1), :]
    return out
